# revision 16
# baseline (speedup 1.0000x reference)
"""Trainium2 Bass kernel for nn_Evaluate (nms_detection).

Contract: kernel(**inputs) takes the FULL unsharded inputs
  pred_masks    [4, 256, 512, 512] f32
  target_masks  [4, 64, 512, 512]  f32
  pred_logits   [4, 256, 81]       f32
  target_clsIds [4, 64]            i32
and returns (precision, recall, accuracy) as float32 scalars, matching
reference.reference().

Sharding: 8 cores; core c handles batch b = c//2, pixel half h = c%2
(hw = 512*512 = 262144 pixels; halves of 131072). Each core computes, on
device, the binarized-mask contraction over its pixel range:
  acc[g, p]   = sum_hw (tgt[g]>0.5) * (pred[p]>0.5)   (intersections)
  acc[64, p]  = sum_hw (pred[p]>0.5)                  (pred_sum)
  acc[g, 256] = sum_hw (tgt[g]>0.5)                   (tgt_sum)
The host adds the two halves per batch, then runs the tiny O(bs*256*64)
greedy NMS matching and the final scalar metrics (identical math to the
reference, in float32).

Device pipeline per core (see build_kernel), processing pixels in pairs of
128-chunks (256 px):
  DMA (two HWDGE rings) loads natural fp32 tiles; the tgt tile is loaded
  pair-stacked [128, f/2] (rows 0:64 = channels at even chunks, 64:128 =
  odd chunks) so ONE [128,128] PE transpose covers both chunks.
  PE transposes raw fp32 into PSUM (4 pred + 1 tgt per pair), DVE is_gt
  binarizes pred PSUM -> bf16 ring tiles [128, 2, 384] (persistent ones
  columns at 256 and 321), ACT copies the tgt block (already 0/1), and one
  bf16 matmul per chunk accumulates acc [65, 257] in a single PSUM bank:
  lhsT = ring[:, ko, 257:322] ([tgtT | ones]), rhs = ring[:, ko, 0:257]
  ([predT | ones]). start=True on a matmul resets its whole PSUM bank, so
  the single accumulation chain owns that bank exclusively.
"""

import os
import sys
from contextlib import ExitStack

import numpy as np

for _p in ("/opt/trn_rl_repo", "/root/.axon_site/_ro/trn_rl_repo"):
    if os.path.isdir(_p) and _p not in sys.path:
        sys.path.insert(0, _p)

from concourse import bacc
import concourse.mybir as mybir
import concourse.tile as tile
from concourse.bass_utils import run_bass_kernel_spmd
from concourse.masks import make_identity

BS = 4
P_CH = 256
G_CH = 64
HW_FULL = 512 * 512
N_CORES = 8
HW = HW_FULL // 2        # pixels per core
F = 4096                 # pixels per DMA tile
CHUNK = 128
PAIR = 256
W = P_CH + G_CH          # 320 binarized data columns per chunk
ONES = W
RING_W = 384             # padded ring width
ONES_R = 256             # ones col used by rhs (tgt_sum / count)
TGT0 = 257               # tgt cols 257:321
ONES_L = 321             # ones col ending lhsT (pred_sum row)

SIZE_THRS = 1.0
CLS_SCORE_THR = 0.5
IOU_THR = 0.5

LAST_EXEC_TIME_NS = None
LAST_TRACE_PATH = None
LAST_ACC = None


def _install_ntff_hook():
    """Register the axon NTFF profiling hook that boot() skips when the
    image's antenv package lacks axon_hooks (see trn_agent_boot.trn_boot)."""
    import types

    try:
        import antenv
    except ImportError:
        return False
    if "antenv.axon_hooks" not in sys.modules:
        mod = types.ModuleType("antenv.axon_hooks")
        mod._hook = None

        def set_axon_ntff_profile_hook(h):
            mod._hook = h

        def get_axon_ntff_profile_hook():
            return mod._hook

        mod.set_axon_ntff_profile_hook = set_axon_ntff_profile_hook
        mod.get_axon_ntff_profile_hook = get_axon_ntff_profile_hook
        sys.modules["antenv.axon_hooks"] = mod
        antenv.axon_hooks = mod
    try:
        from antenv.axon_hooks import get_axon_ntff_profile_hook, set_axon_ntff_profile_hook

        if get_axon_ntff_profile_hook() is None:
            from trn_agent_boot.trn_boot import _ntff_profile_via_ctypes

            hook = _ntff_profile_via_ctypes("/opt/axon/libaxon_pjrt.so")
            if hook is None:
                return False
            set_axon_ntff_profile_hook(hook)
        return True
    except Exception:
        return False


def build_kernel(hw: int = HW, f: int = F, nbuf: int = 6, psum_bufs: int = 3):
    assert f % PAIR == 0 and hw % f == 0
    nc = bacc.Bacc("TRN2", target_bir_lowering=False)

    pred = nc.dram_tensor("pred", [P_CH, hw], mybir.dt.float32, kind="ExternalInput")
    tgt = nc.dram_tensor("tgt", [G_CH, hw], mybir.dt.float32, kind="ExternalInput")
    out = nc.dram_tensor("acc", [G_CH + 1, P_CH + 1], mybir.dt.float32, kind="ExternalOutput")

    n_tiles = hw // f
    pairs_per_tile = f // PAIR
    total_pairs = hw // PAIR

    with ExitStack() as ctx:
        tc = ctx.enter_context(tile.TileContext(nc))
        nat_pool = ctx.enter_context(tc.tile_pool(name="nat", bufs=3))
        bin_pool = ctx.enter_context(tc.tile_pool(name="bin", bufs=2))
        tpP_pool = ctx.enter_context(tc.tile_pool(name="tpP", bufs=psum_bufs, space="PSUM"))
        tpT_pool = ctx.enter_context(tc.tile_pool(name="tpT", bufs=psum_bufs, space="PSUM"))
        acc_pool = ctx.enter_context(tc.tile_pool(name="accp", bufs=1, space="PSUM"))
        tsb_pool = ctx.enter_context(tc.tile_pool(name="tsb", bufs=1))
        misc_pool = ctx.enter_context(tc.tile_pool(name="misc", bufs=1))

        identb = misc_pool.tile([128, 128], mybir.dt.bfloat16)
        make_identity(nc, identb)
        identf = misc_pool.tile([128, 128], mybir.dt.float32)
        make_identity(nc, identf)

        acc = acc_pool.tile([G_CH + 1, P_CH + 1], mybir.dt.float32)

        ring = []
        for r in range(nbuf):
            tb = tsb_pool.tile([128, 2, RING_W], mybir.dt.float8e4, tag=f"ring{r}")
            nc.vector.memset(tb[:, :, ONES_R : ONES_R + 1], 1.0)
            nc.vector.memset(tb[:, :, ONES_L : ONES_L + 1], 1.0)
            ring.append(tb)

        ci = 0
        for t in range(n_tiles):
            pred_lo = nat_pool.tile([128, f], mybir.dt.float32, tag="pred_lo")
            pred_hi = nat_pool.tile([128, f], mybir.dt.float32, tag="pred_hi")
            tgt_nat = nat_pool.tile([128, f // 2], mybir.dt.float32, tag="tgt_nat")
            sl = slice(t * f, (t + 1) * f)
            nc.sync.dma_start(out=pred_lo, in_=pred[0:128, sl])
            nc.scalar.dma_start(out=pred_hi, in_=pred[128:256, sl])
            tgt_src = tgt[:, sl].rearrange("c (q two k) -> c q two k", two=2, k=CHUNK)
            tgt_dst = tgt_nat.rearrange("c (q k) -> c q k", k=CHUNK)
            nc.sync.dma_start(out=tgt_dst[0:G_CH], in_=tgt_src[:, :, 0, :])
            nc.scalar.dma_start(out=tgt_dst[G_CH:128], in_=tgt_src[:, :, 1, :])

            bin_lo = bin_pool.tile([128, f], mybir.dt.bfloat16, tag="bin_lo")
            bin_hi = bin_pool.tile([128, f], mybir.dt.bfloat16, tag="bin_hi")
            nc.vector.tensor_scalar(out=bin_lo, in0=pred_lo, scalar1=0.5, scalar2=None,
                                    op0=mybir.AluOpType.is_gt)
            nc.vector.tensor_scalar(out=bin_hi, in0=pred_hi, scalar1=0.5, scalar2=None,
                                    op0=mybir.AluOpType.is_gt)

            for q in range(pairs_per_tile):
                s0 = slice(2 * q * CHUNK, (2 * q + 1) * CHUNK)
                s1 = slice((2 * q + 1) * CHUNK, (2 * q + 2) * CHUNK)
                sq = slice(q * CHUNK, (q + 1) * CHUNK)
                tpP = tpP_pool.tile([128, 512], mybir.dt.bfloat16, tag="tpP")
                tpT = tpT_pool.tile([128, 128], mybir.dt.float32, tag="tpT")
                nc.tensor.transpose(tpP[:, 0:128], bin_lo[:, s0], identb)
                nc.tensor.transpose(tpP[:, 128:256], bin_hi[:, s0], identb)
                nc.tensor.transpose(tpP[:, 256:384], bin_lo[:, s1], identb)
                nc.tensor.transpose(tpP[:, 384:512], bin_hi[:, s1], identb)
                nc.tensor.transpose(tpT, tgt_nat[:, sq], identf)

                tb = ring[ci % nbuf]
                tpPv = tpP.rearrange("p (two n) -> p two n", two=2)
                nc.vector.tensor_copy(out=tb[:, 0, 0:P_CH], in_=tpPv[:, 0])
                nc.scalar.activation(
                    out=tb[:, 1, 0:P_CH], in_=tpPv[:, 1],
                    func=mybir.ActivationFunctionType.Copy,
                )
                nc.scalar.activation(
                    out=tb[:, :, TGT0 : TGT0 + G_CH],
                    in_=tpT.rearrange("p (two g) -> p two g", two=2),
                    func=mybir.ActivationFunctionType.Copy,
                )

                nc.tensor.matmul(
                    acc, lhsT=tb[:, :, TGT0 : ONES_L + 1],
                    rhs=tb[:, :, 0 : ONES_R + 1],
                    start=(ci == 0), stop=(ci == total_pairs - 1),
                    perf_mode=mybir.MatmulPerfMode.DoubleRow,
                )
                ci += 1

        acc_sb = misc_pool.tile([G_CH + 1, P_CH + 1], mybir.dt.float32)
        nc.vector.tensor_copy(out=acc_sb, in_=acc)
        nc.sync.dma_start(out=out[:, :], in_=acc_sb)

    nc.finalize()
    return nc


_NC_CACHE = None


def _get_nc():
    global _NC_CACHE
    if _NC_CACHE is None:
        _NC_CACHE = build_kernel()
    return _NC_CACHE


def _run_device(pred_masks: np.ndarray, target_masks: np.ndarray):
    """Run the 8-core SPMD kernel; returns acc [BS, 65, 257] f32 (halves
    already summed per batch)."""
    global LAST_EXEC_TIME_NS, LAST_TRACE_PATH
    nc = _get_nc()

    pred_flat = np.ascontiguousarray(pred_masks.reshape(BS, P_CH, 2, HW).transpose(0, 2, 1, 3))
    tgt_flat = np.ascontiguousarray(target_masks.reshape(BS, G_CH, 2, HW).transpose(0, 2, 1, 3))
    in_maps = []
    for c in range(N_CORES):
        b, h = divmod(c, 2)
        in_maps.append({"pred": pred_flat[b, h], "tgt": tgt_flat[b, h]})

    trace = bool(int(os.environ.get("KERNEL_TRACE", "0")))
    if trace:
        trace = _install_ntff_hook()
    kw = dict(trace=True) if trace else {}
    try:
        res = run_bass_kernel_spmd(nc, in_maps, core_ids=list(range(N_CORES)), **kw)
    except Exception:
        if not trace:
            raise
        res = run_bass_kernel_spmd(nc, in_maps, core_ids=list(range(N_CORES)))
    LAST_EXEC_TIME_NS = res.exec_time_ns
    if res.instructions_and_trace is not None:
        LAST_TRACE_PATH = res.instructions_and_trace[1]

    acc = np.zeros((BS, G_CH + 1, P_CH + 1), np.float64)
    for c in range(N_CORES):
        b = c // 2
        acc[b] += res.results[c]["acc"].astype(np.float64)
    global LAST_ACC
    LAST_ACC = acc
    return acc


def _greedy_match(iou, score, cls, psum, tcls):
    """Faithful numpy replica of reference._greedy_match (one batch)."""
    order = np.argsort(-score, kind="stable")
    iou_m = iou.copy()
    tp = 0.0
    fp = 0.0
    for pk in order:
        skip = (cls[pk] == 0) or (psum[pk] < SIZE_THRS) or (score[pk] < CLS_SCORE_THR)
        row = iou_m[pk]
        gk = int(np.argmax(row))
        hit = (row[gk] >= IOU_THR) and (cls[pk] == tcls[gk]) and (not skip)
        if hit:
            tp += 1.0
            iou_m[:, gk] = 0.0
        elif not skip:
            fp += 1.0
    return np.float32(tp), np.float32(fp)


def kernel(pred_masks, target_masks, pred_logits, target_clsIds):
    pred_masks = np.asarray(pred_masks, dtype=np.float32)
    target_masks = np.asarray(target_masks, dtype=np.float32)
    pred_logits = np.asarray(pred_logits, dtype=np.float32)
    target_clsIds = np.asarray(target_clsIds, dtype=np.int32)

    acc = _run_device(pred_masks, target_masks)

    # Host epilogue (tiny): iou + scores + greedy matching, all float32 math
    # mirroring the reference.
    intp = acc[:, 0:G_CH, 0:P_CH].transpose(0, 2, 1).astype(np.float32)  # [b, p, g]
    pred_sum = acc[:, G_CH, 0:P_CH].astype(np.float32)                   # [b, p]
    tgt_sum = acc[:, 0:G_CH, P_CH].astype(np.float32)                    # [b, g]

    union = pred_sum[:, :, None] + tgt_sum[:, None, :] - intp
    iou = intp / (union + np.float32(0.01))

    # softmax scores and argmax classes (fp32, same formula as jax.nn.softmax)
    m = pred_logits.max(axis=-1, keepdims=True)
    e = np.exp(pred_logits - m)
    sm = e / e.sum(axis=-1, keepdims=True)
    score = sm.max(axis=-1).astype(np.float32)                            # [b, p]
    cls = pred_logits.argmax(axis=-1).astype(np.int32)                    # [b, p]

    tp = np.float32(0.0)
    fp = np.float32(0.0)
    for b in range(BS):
        tp_b, fp_b = _greedy_match(iou[b], score[b], cls[b], pred_sum[b], target_clsIds[b])
        tp += tp_b
        fp += fp_b

    tot_target = np.float32((target_clsIds > 0).sum())
    precision = tp / (tp + fp + np.float32(0.001))
    recall = tp / (tot_target + np.float32(0.001))
    accuracy = tp / (tot_target + fp + np.float32(0.001))
    return (np.float32(precision), np.float32(recall), np.float32(accuracy))


# revision 17
# speedup vs baseline: 1.2677x; 1.2677x over previous
"""Trainium2 Bass kernel for nn_Evaluate (nms_detection).

Contract: kernel(**inputs) takes the FULL unsharded inputs
  pred_masks    [4, 256, 512, 512] f32
  target_masks  [4, 64, 512, 512]  f32
  pred_logits   [4, 256, 81]       f32
  target_clsIds [4, 64]            i32
and returns (precision, recall, accuracy) as float32 scalars, matching
reference.reference().

Sharding: 8 cores; core c handles batch b = c//2, pixel half h = c%2
(hw = 512*512 = 262144 pixels; halves of 131072). Each core computes, on
device, the binarized-mask contraction over its pixel range:
  acc[g, p]   = sum_hw (tgt[g]>0.5) * (pred[p]>0.5)   (intersections)
  acc[64, p]  = sum_hw (pred[p]>0.5)                  (pred_sum)
  acc[g, 256] = sum_hw (tgt[g]>0.5)                   (tgt_sum)
The host adds the two halves per batch, then runs the tiny O(bs*256*64)
greedy NMS matching and the final scalar metrics (identical math to the
reference, in float32).

Device pipeline per core (see build_kernel), processing pixels in pairs of
128-chunks (256 px):
  DMA (two HWDGE rings) loads natural fp32 tiles; the tgt tile is loaded
  pair-stacked [128, f/2] (rows 0:64 = channels at even chunks, 64:128 =
  odd chunks) so ONE [128,128] PE transpose covers both chunks.
  PE transposes raw fp32 into PSUM (4 pred + 1 tgt per pair), DVE is_gt
  binarizes pred PSUM -> bf16 ring tiles [128, 2, 384] (persistent ones
  columns at 256 and 321), ACT copies the tgt block (already 0/1), and one
  bf16 matmul per chunk accumulates acc [65, 257] in a single PSUM bank:
  lhsT = ring[:, ko, 257:322] ([tgtT | ones]), rhs = ring[:, ko, 0:257]
  ([predT | ones]). start=True on a matmul resets its whole PSUM bank, so
  the single accumulation chain owns that bank exclusively.
"""

import os
import sys
from contextlib import ExitStack

import numpy as np

for _p in ("/opt/trn_rl_repo", "/root/.axon_site/_ro/trn_rl_repo"):
    if os.path.isdir(_p) and _p not in sys.path:
        sys.path.insert(0, _p)

from concourse import bacc
import concourse.mybir as mybir
import concourse.tile as tile
from concourse.bass_utils import run_bass_kernel_spmd
from concourse.masks import make_identity

BS = 4
P_CH = 256
G_CH = 64
HW_FULL = 512 * 512
N_CORES = 8
HW = HW_FULL // 2        # pixels per core
F = 4096                 # pixels per DMA tile
CHUNK = 128
PAIR = 256
W = P_CH + G_CH          # 320 binarized data columns per chunk
ONES = W
RING_W = 384             # padded ring width
ONES_R = 256             # ones col used by rhs (tgt_sum / count)
TGT0 = 257               # tgt cols 257:321
ONES_L = 321             # ones col ending lhsT (pred_sum row)

SIZE_THRS = 1.0
CLS_SCORE_THR = 0.5
IOU_THR = 0.5

LAST_EXEC_TIME_NS = None
LAST_TRACE_PATH = None
LAST_ACC = None


def _install_ntff_hook():
    """Register the axon NTFF profiling hook that boot() skips when the
    image's antenv package lacks axon_hooks (see trn_agent_boot.trn_boot)."""
    import types

    try:
        import antenv
    except ImportError:
        return False
    if "antenv.axon_hooks" not in sys.modules:
        mod = types.ModuleType("antenv.axon_hooks")
        mod._hook = None

        def set_axon_ntff_profile_hook(h):
            mod._hook = h

        def get_axon_ntff_profile_hook():
            return mod._hook

        mod.set_axon_ntff_profile_hook = set_axon_ntff_profile_hook
        mod.get_axon_ntff_profile_hook = get_axon_ntff_profile_hook
        sys.modules["antenv.axon_hooks"] = mod
        antenv.axon_hooks = mod
    try:
        from antenv.axon_hooks import get_axon_ntff_profile_hook, set_axon_ntff_profile_hook

        if get_axon_ntff_profile_hook() is None:
            from trn_agent_boot.trn_boot import _ntff_profile_via_ctypes

            hook = _ntff_profile_via_ctypes("/opt/axon/libaxon_pjrt.so")
            if hook is None:
                return False
            set_axon_ntff_profile_hook(hook)
        return True
    except Exception:
        return False


def build_kernel(hw: int = HW, f: int = F, nbuf: int = 8, psum_bufs: int = 3):
    assert f % PAIR == 0 and hw % f == 0
    nc = bacc.Bacc("TRN2", target_bir_lowering=False)

    pred = nc.dram_tensor("pred", [P_CH, hw], mybir.dt.float32, kind="ExternalInput")
    tgt = nc.dram_tensor("tgt", [G_CH, hw], mybir.dt.float32, kind="ExternalInput")
    out = nc.dram_tensor("acc", [G_CH + 1, P_CH + 1], mybir.dt.float32, kind="ExternalOutput")

    n_tiles = hw // f
    pairs_per_tile = f // PAIR
    total_pairs = hw // PAIR

    with ExitStack() as ctx:
        tc = ctx.enter_context(tile.TileContext(nc))
        nat_pool = ctx.enter_context(tc.tile_pool(name="nat", bufs=4))
        tpP_pool = ctx.enter_context(tc.tile_pool(name="tpP", bufs=4, space="PSUM"))
        tpT_pool = ctx.enter_context(tc.tile_pool(name="tpT", bufs=psum_bufs, space="PSUM"))
        acc_pool = ctx.enter_context(tc.tile_pool(name="accp", bufs=1, space="PSUM"))
        tsb_pool = ctx.enter_context(tc.tile_pool(name="tsb", bufs=1))
        misc_pool = ctx.enter_context(tc.tile_pool(name="misc", bufs=1))

        ident = misc_pool.tile([128, 128], mybir.dt.float32)
        make_identity(nc, ident)

        acc = acc_pool.tile([G_CH + 1, P_CH + 1], mybir.dt.float32)

        ring = []
        for r in range(nbuf):
            tb = tsb_pool.tile([128, 2, RING_W], mybir.dt.bfloat16, tag=f"ring{r}")
            nc.vector.memset(tb[:, :, ONES_R : ONES_R + 1], 1.0)
            nc.vector.memset(tb[:, :, ONES_L : ONES_L + 1], 1.0)
            ring.append(tb)

        ci = 0
        for t in range(n_tiles):
            pred_lo = nat_pool.tile([128, f], mybir.dt.float32, tag="pred_lo")
            pred_hi = nat_pool.tile([128, f], mybir.dt.float32, tag="pred_hi")
            # tgt pair-stacked: [128, f/2]; viewed as [128, pairs, 128]
            tgt_nat = nat_pool.tile([128, f // 2], mybir.dt.float32, tag="tgt_nat")
            sl = slice(t * f, (t + 1) * f)
            nc.sync.dma_start(out=pred_lo, in_=pred[0:128, sl])
            nc.scalar.dma_start(out=pred_hi, in_=pred[128:256, sl])
            tgt_src = tgt[:, sl].rearrange("c (q two k) -> c q two k", two=2, k=CHUNK)
            tgt_dst = tgt_nat.rearrange("c (q k) -> c q k", k=CHUNK)
            nc.sync.dma_start(out=tgt_dst[0:G_CH], in_=tgt_src[:, :, 0, :])
            nc.sync.dma_start(out=tgt_dst[G_CH:128], in_=tgt_src[:, :, 1, :])

            for q in range(pairs_per_tile):
                s0 = slice(2 * q * CHUNK, (2 * q + 1) * CHUNK)
                s1 = slice((2 * q + 1) * CHUNK, (2 * q + 2) * CHUNK)
                sq = slice(q * CHUNK, (q + 1) * CHUNK)
                tpP = tpP_pool.tile([128, 512], mybir.dt.float32, tag="tpP")
                tpT = tpT_pool.tile([128, 128], mybir.dt.float32, tag="tpT")
                nc.tensor.transpose(tpP[:, 0:128], pred_lo[:, s0], ident)
                nc.tensor.transpose(tpP[:, 128:256], pred_hi[:, s0], ident)
                nc.tensor.transpose(tpP[:, 256:384], pred_lo[:, s1], ident)
                nc.tensor.transpose(tpP[:, 384:512], pred_hi[:, s1], ident)
                nc.tensor.transpose(tpT, tgt_nat[:, sq], ident)

                tb = ring[ci % nbuf]
                # one is_gt per pair: [128, 2, 256] strided dst
                nc.vector.tensor_scalar(
                    out=tb[:, :, 0:P_CH],
                    in0=tpP.rearrange("p (two n) -> p two n", two=2),
                    scalar1=0.5, scalar2=None, op0=mybir.AluOpType.is_gt,
                )
                nc.scalar.activation(
                    out=tb[:, :, TGT0 : TGT0 + G_CH],
                    in_=tpT.rearrange("p (two g) -> p two g", two=2),
                    func=mybir.ActivationFunctionType.Copy,
                )

                for ko in range(2):
                    nc.tensor.matmul(
                        acc, lhsT=tb[:, ko, TGT0 : ONES_L + 1],
                        rhs=tb[:, ko, 0 : ONES_R + 1],
                        start=(ci == 0 and ko == 0),
                        stop=(ci == total_pairs - 1 and ko == 1),
                    )
                ci += 1

        acc_sb = misc_pool.tile([G_CH + 1, P_CH + 1], mybir.dt.float32)
        nc.vector.tensor_copy(out=acc_sb, in_=acc)
        nc.sync.dma_start(out=out[:, :], in_=acc_sb)

    nc.finalize()
    return nc


_NC_CACHE = None


def _get_nc():
    global _NC_CACHE
    if _NC_CACHE is None:
        _NC_CACHE = build_kernel()
    return _NC_CACHE


def _run_device(pred_masks: np.ndarray, target_masks: np.ndarray):
    """Run the 8-core SPMD kernel; returns acc [BS, 65, 257] f32 (halves
    already summed per batch)."""
    global LAST_EXEC_TIME_NS, LAST_TRACE_PATH
    nc = _get_nc()

    pred_flat = np.ascontiguousarray(pred_masks.reshape(BS, P_CH, 2, HW).transpose(0, 2, 1, 3))
    tgt_flat = np.ascontiguousarray(target_masks.reshape(BS, G_CH, 2, HW).transpose(0, 2, 1, 3))
    in_maps = []
    for c in range(N_CORES):
        b, h = divmod(c, 2)
        in_maps.append({"pred": pred_flat[b, h], "tgt": tgt_flat[b, h]})

    trace = bool(int(os.environ.get("KERNEL_TRACE", "0")))
    if trace:
        trace = _install_ntff_hook()
    kw = dict(trace=True) if trace else {}
    try:
        res = run_bass_kernel_spmd(nc, in_maps, core_ids=list(range(N_CORES)), **kw)
    except Exception:
        if not trace:
            raise
        res = run_bass_kernel_spmd(nc, in_maps, core_ids=list(range(N_CORES)))
    LAST_EXEC_TIME_NS = res.exec_time_ns
    if res.instructions_and_trace is not None:
        LAST_TRACE_PATH = res.instructions_and_trace[1]

    acc = np.zeros((BS, G_CH + 1, P_CH + 1), np.float64)
    for c in range(N_CORES):
        b = c // 2
        acc[b] += res.results[c]["acc"].astype(np.float64)
    global LAST_ACC
    LAST_ACC = acc
    return acc


def _greedy_match(iou, score, cls, psum, tcls):
    """Faithful numpy replica of reference._greedy_match (one batch)."""
    order = np.argsort(-score, kind="stable")
    iou_m = iou.copy()
    tp = 0.0
    fp = 0.0
    for pk in order:
        skip = (cls[pk] == 0) or (psum[pk] < SIZE_THRS) or (score[pk] < CLS_SCORE_THR)
        row = iou_m[pk]
        gk = int(np.argmax(row))
        hit = (row[gk] >= IOU_THR) and (cls[pk] == tcls[gk]) and (not skip)
        if hit:
            tp += 1.0
            iou_m[:, gk] = 0.0
        elif not skip:
            fp += 1.0
    return np.float32(tp), np.float32(fp)


def kernel(pred_masks, target_masks, pred_logits, target_clsIds):
    pred_masks = np.asarray(pred_masks, dtype=np.float32)
    target_masks = np.asarray(target_masks, dtype=np.float32)
    pred_logits = np.asarray(pred_logits, dtype=np.float32)
    target_clsIds = np.asarray(target_clsIds, dtype=np.int32)

    acc = _run_device(pred_masks, target_masks)

    # Host epilogue (tiny): iou + scores + greedy matching, all float32 math
    # mirroring the reference.
    intp = acc[:, 0:G_CH, 0:P_CH].transpose(0, 2, 1).astype(np.float32)  # [b, p, g]
    pred_sum = acc[:, G_CH, 0:P_CH].astype(np.float32)                   # [b, p]
    tgt_sum = acc[:, 0:G_CH, P_CH].astype(np.float32)                    # [b, g]

    union = pred_sum[:, :, None] + tgt_sum[:, None, :] - intp
    iou = intp / (union + np.float32(0.01))

    # softmax scores and argmax classes (fp32, same formula as jax.nn.softmax)
    m = pred_logits.max(axis=-1, keepdims=True)
    e = np.exp(pred_logits - m)
    sm = e / e.sum(axis=-1, keepdims=True)
    score = sm.max(axis=-1).astype(np.float32)                            # [b, p]
    cls = pred_logits.argmax(axis=-1).astype(np.int32)                    # [b, p]

    tp = np.float32(0.0)
    fp = np.float32(0.0)
    for b in range(BS):
        tp_b, fp_b = _greedy_match(iou[b], score[b], cls[b], pred_sum[b], target_clsIds[b])
        tp += tp_b
        fp += fp_b

    tot_target = np.float32((target_clsIds > 0).sum())
    precision = tp / (tp + fp + np.float32(0.001))
    recall = tp / (tot_target + np.float32(0.001))
    accuracy = tp / (tot_target + fp + np.float32(0.001))
    return (np.float32(precision), np.float32(recall), np.float32(accuracy))
